# revision 7
# baseline (speedup 1.0000x reference)
"""Trainium2 Bass kernel for nn_IntegratedLaughterModel.

Strategy (pure data parallel, 8 samples/core):
  - Algebraic reduction: scores[b,h,s] = x[b,s,:] @ qk[:,h] where
    qk = (Wk reshaped) @ q_tom / sqrt(DH)  (host-precomputed [D, NH]).
    This removes the two [S,D]x[D,D] matmuls entirely.
  - Single pass over x per core: per 512-token group,
      PE-transpose x chunks -> scores matmul (fp32r) -> mask via rank-4
      log-mask matmul -> ACT exp (with accum Z) -> PE-transpose weights ->
      pooling matmul accumulating [11, D] per sample.
    rows 0..7  = unnormalized attn-weighted sums per head,
    rows 8..10 = masked means (mean/setup/punch; masks pre-scaled on host,
                 folded in as exp(log(mask)) rows so one pooling matmul).
  - Small per-core head (feature-major, [128d, 8b] tiles) computes the
    ToM/GCACU/CLoST/mHC/SEVADE/final MLPs on-device.
"""

import os
import numpy as np

B, S, D, HID, NH = 64, 2048, 512, 512, 8
DH = D // NH
NCORES = 8
BPC = B // NCORES   # samples per core
NG = 4              # 512-token groups per sample
GT = 512            # tokens per group
NT = 4              # 128-token subtiles per group
NCD = 4             # d-chunks of 128
EPS = 1e-4
MASK_NEG = -30.0    # additive penalty for masked tokens (exp(-30) ~ 1e-13)
LOG_FLOOR = -80.0

F32R = os.environ.get("KERNEL_F32R", "1") == "1"

_CACHE = {}
LAST_RESULT = None


def _build_program():
    import concourse.bacc as bacc
    import concourse.tile as tile
    from concourse import mybir
    from contextlib import ExitStack

    f32 = mybir.dt.float32
    bf16 = mybir.dt.bfloat16
    AF = mybir.ActivationFunctionType
    ALU = mybir.AluOpType

    nc = bacc.Bacc("TRN2", target_bir_lowering=False, debug=False,
                   enable_asserts=False)

    # ---- DRAM I/O ----
    x_d = nc.dram_tensor("x", [BPC, S, D], bf16, kind="ExternalInput").ap()
    rhs4_d = nc.dram_tensor("rhs4", [4, BPC * NG * GT], bf16, kind="ExternalInput").ap()
    qk_d = nc.dram_tensor("qk", [128, NCD * 11], bf16, kind="ExternalInput").ap()
    sel_d = nc.dram_tensor("sel", [4, 11], bf16, kind="ExternalInput").ap()
    id_d = nc.dram_tensor("ident", [128, 128], bf16, kind="ExternalInput").ap()
    id32_d = nc.dram_tensor("ident32", [16, 16], f32, kind="ExternalInput").ap()
    wv_d = nc.dram_tensor("wv", [128, 2048], f32, kind="ExternalInput").ap()
    wtf_d = nc.dram_tensor("wtf", [128, 2048], f32, kind="ExternalInput").ap()
    wg1_d = nc.dram_tensor("wg1", [128, 2048], f32, kind="ExternalInput").ap()
    wg2_d = nc.dram_tensor("wg2", [128, 2048], f32, kind="ExternalInput").ap()
    wc1_d = nc.dram_tensor("wc1", [128, 4096], f32, kind="ExternalInput").ap()
    ws1_d = nc.dram_tensor("ws1", [128, 2048], f32, kind="ExternalInput").ap()
    ws1t_d = nc.dram_tensor("ws1t", [3, 512], f32, kind="ExternalInput").ap()
    wf1_d = nc.dram_tensor("wf1", [128, 2048], f32, kind="ExternalInput").ap()
    wf1t_d = nc.dram_tensor("wf1t", [3, 512], f32, kind="ExternalInput").ap()
    vecs_d = nc.dram_tensor("vecs", [128, 20], f32, kind="ExternalInput").ap()
    bvecs_d = nc.dram_tensor("bvecs", [128, 20], f32, kind="ExternalInput").ap()
    b5_d = nc.dram_tensor("b5", [1, 5], f32, kind="ExternalInput").ap()
    m3_d = nc.dram_tensor("m3", [1, 9], f32, kind="ExternalInput").ap()
    out_d = nc.dram_tensor("out", [1, BPC], f32, kind="ExternalOutput").ap()
    diag_d = nc.dram_tensor("diag", [BPC * 11, D], f32, kind="ExternalOutput").ap()

    with tile.TileContext(nc) as tc, ExitStack() as ctx:
        cst = ctx.enter_context(tc.tile_pool(name="cst", bufs=1))

        def static(name, shape, src, dt=f32):
            t = cst.tile(shape, dt, tag=name, name=name)
            nc.sync.dma_start(out=t[:], in_=src)
            return t

        qk_sb = static("qk", [128, NCD * 11], qk_d, bf16)
        sel_sb = static("sel", [4, 11], sel_d, bf16)
        id_sb = static("ident", [128, 128], id_d, bf16)
        id32_sb = static("ident32", [16, 16], id32_d, f32)
        rhs4_sb = static("rhs4", [4, BPC * NG * GT], rhs4_d, bf16)
        ones_sb = cst.tile([128, 1], f32, tag="ones")
        nc.vector.memset(ones_sb[:], 1.0)
        ones16_sb = cst.tile([128, 1], bf16, tag="ones16")
        nc.vector.memset(ones16_sb[:], 1.0)

        # pooledT chunks: [128 d, BPC*11] feature-major pooled quantities
        pT = [cst.tile([128, BPC * 11], f32, tag=f"pT{c}", name=f"pT{c}")
              for c in range(NCD)]

        H = {}

        def load_head_weights():
            H["wv"] = static("wv", [128, 2048], wv_d)
            H["wtf"] = static("wtf", [128, 2048], wtf_d)
            H["wg1"] = static("wg1", [128, 2048], wg1_d)
            H["wg2"] = static("wg2", [128, 2048], wg2_d)
            H["wc1"] = static("wc1", [128, 4096], wc1_d)
            H["ws1"] = static("ws1", [128, 2048], ws1_d)
            H["ws1t"] = static("ws1t", [3, 512], ws1t_d)
            H["wf1"] = static("wf1", [128, 2048], wf1_d)
            H["wf1t"] = static("wf1t", [3, 512], wf1t_d)
            H["vecs"] = static("vecs", [128, 20], vecs_d)
            H["bvecs"] = static("bvecs", [128, 20], bvecs_d)
            H["b5"] = static("b5", [1, 5], b5_d)
            H["m3"] = static("m3", [1, 9], m3_d)

        # ================= main streaming pass =================
        with ExitStack() as pctx:
            xg_p = pctx.enter_context(tc.tile_pool(name="xg", bufs=3))
            xt_ps_p = pctx.enter_context(tc.tile_pool(name="xtps", bufs=2, space="PSUM"))
            xt_sb_p = pctx.enter_context(tc.tile_pool(name="xtsb", bufs=2))
            sc_ps_p = pctx.enter_context(tc.tile_pool(name="scps", bufs=2, space="PSUM"))
            w_sb_p = pctx.enter_context(tc.tile_pool(name="wsb", bufs=2))
            wt_ps_p = pctx.enter_context(tc.tile_pool(name="wtps", bufs=1, space="PSUM"))
            wt_sb_p = pctx.enter_context(tc.tile_pool(name="wtsb", bufs=2))
            pool_ps_p = pctx.enter_context(tc.tile_pool(name="poolps", bufs=2, space="PSUM"))
            z_ps_p = pctx.enter_context(tc.tile_pool(name="zps", bufs=1, space="PSUM"))
            small_p = pctx.enter_context(tc.tile_pool(name="small", bufs=2))

            for b in range(BPC):
                pool_ps = pool_ps_p.tile([11, D], f32, tag="pool")
                z_ps = z_ps_p.tile([11, 1], f32, tag="z")
                for g in range(NG):
                    xg = xg_p.tile([128, NT * D], bf16, tag="xg")
                    src = x_d[b, g * GT:(g + 1) * GT, :].rearrange(
                        "(t p) d -> p t d", p=128)
                    nc.sync.dma_start(
                        out=xg[:].rearrange("p (t d) -> p t d", d=D), in_=src)

                    sc_ps = sc_ps_p.tile([11, GT], f32, tag="sc")
                    # transpose x chunks and accumulate scores over d-chunks
                    for c in range(NCD):
                        xt_ps = xt_ps_p.tile([128, GT], bf16, tag="xt")
                        for t in range(NT):
                            nc.tensor.transpose(
                                xt_ps[:, t * 128:(t + 1) * 128],
                                xg[:, t * D + c * 128: t * D + (c + 1) * 128],
                                id_sb[:])
                        xt_sb = xt_sb_p.tile([128, GT], bf16, tag="xt")
                        nc.vector.tensor_copy(xt_sb[:], xt_ps[:])
                        nc.tensor.matmul(
                            sc_ps[:], qk_sb[:, c * 11:(c + 1) * 11],
                            xt_sb[:], start=(c == 0), stop=False)
                    # rank-4 mask/log-mask rows
                    col0 = (b * NG + g) * GT
                    nc.tensor.matmul(
                        sc_ps[:], sel_sb[:],
                        rhs4_sb[:, col0:col0 + GT], start=False, stop=True)
                    # exp
                    w_sb = w_sb_p.tile([11, GT], f32, tag="w")
                    nc.scalar.activation(w_sb[:], sc_ps[:], AF.Exp)
                    # transpose w -> [128, 11] per subtile (cast to bf16); pool
                    wt_ps = wt_ps_p.tile([128, NT * 11], f32, tag="wt")
                    for t in range(NT):
                        nc.tensor.transpose(
                            wt_ps[:, t * 11:(t + 1) * 11],
                            w_sb[:, t * 128:(t + 1) * 128],
                            id32_sb[:11, :11])
                    wt_sb = wt_sb_p.tile([128, NT * 11], bf16, tag="wt")
                    nc.vector.tensor_copy(wt_sb[:], wt_ps[:])
                    for t in range(NT):
                        nc.tensor.matmul(
                            pool_ps[:], wt_sb[:, t * 11:(t + 1) * 11],
                            xg[:, t * D:(t + 1) * D],
                            start=(g == 0 and t == 0),
                            stop=(g == NG - 1 and t == NT - 1))
                        nc.tensor.matmul(
                            z_ps[:], wt_sb[:, t * 11:(t + 1) * 11],
                            ones16_sb[:],
                            start=(g == 0 and t == 0),
                            stop=(g == NG - 1 and t == NT - 1))

                # normalize by Z and extract feature-major pooledT
                zr = small_p.tile([11, 1], f32, tag="zr")
                nc.vector.reciprocal(zr[:], z_ps[:])
                pooled_sb = small_p.tile([11, D], f32, tag="pooled")
                nc.vector.tensor_scalar_mul(pooled_sb[:], pool_ps[:], zr[:])
                nc.sync.dma_start(out=diag_d[b * 11:(b + 1) * 11, :],
                                  in_=pooled_sb[:])
                for c in range(NCD):
                    pt_ps = wt_ps_p.tile([128, NT * 11], f32, tag="wt")
                    nc.tensor.transpose(pt_ps[:, 0:11],
                                        pooled_sb[:, c * 128:(c + 1) * 128],
                                        id32_sb[:11, :11])
                    nc.vector.tensor_copy(pT[c][:, b * 11:(b + 1) * 11],
                                          pt_ps[:, 0:11])

                if b == 0:
                    load_head_weights()

        # ================= head (feature-major, all 8 samples) =================
        def cview(c, r):
            """[128, 8] view of quantity r across samples in pooledT chunk c."""
            return pT[c][:].rearrange("p (b q) -> p b q", q=11)[:, :, r]

        with ExitStack() as hctx:
            pj = hctx.enter_context(tc.tile_pool(name="pj", bufs=2, space="PSUM"))
            ptiny = hctx.enter_context(tc.tile_pool(name="ptiny", bufs=1, space="PSUM"))
            hp = hctx.enter_context(tc.tile_pool(name="hp", bufs=1))
            htmp = hctx.enter_context(tc.tile_pool(name="htmp", bufs=4))

            wv = H["wv"]; wtf = H["wtf"]; wg1 = H["wg1"]; wg2 = H["wg2"]
            wc1 = H["wc1"]; ws1 = H["ws1"]; ws1t = H["ws1t"]; wf1 = H["wf1"]
            wf1t = H["wf1t"]; vecs = H["vecs"]; bvecs = H["bvecs"]
            b5 = H["b5"]; m3 = H["m3"]

            def vcol(k, c):
                return vecs[:, k * 4 + c: k * 4 + c + 1]

            def bcol(k, c):
                return bvecs[:, k * 4 + c: k * 4 + c + 1]

            # ---- fusedT = blockdiag(Wv) applied to attn-pooled heads ----
            fused_sb = []
            for i in range(4):
                ps = pj.tile([128, BPC], f32, tag="proj")
                for hh in range(2):
                    h = 2 * i + hh
                    o = ps[hh * 64:(hh + 1) * 64, :]
                    for c in range(NCD):
                        nc.tensor.matmul(
                            o, wv[:, c * D + h * DH: c * D + (h + 1) * DH],
                            cview(c, h), start=(c == 0), stop=(c == NCD - 1))
                t = hp.tile([128, BPC], f32, tag=f"fused{i}")
                nc.vector.tensor_copy(t[:], ps[:])
                fused_sb.append(t)

            def proj512(w_tile, rhs_aps, consume, nchunks=4):
                """per jc: ps[j,b] = sum_c W_chunk.T @ rhs_c; consume(jc, ps)."""
                outs = []
                for jc in range(4):
                    ps = pj.tile([128, BPC], f32, tag="proj")
                    for c in range(nchunks):
                        nc.tensor.matmul(
                            ps[:],
                            w_tile[:, c * D + jc * 128: c * D + jc * 128 + 128],
                            rhs_aps[c], start=(c == 0), stop=(c == nchunks - 1))
                    outs.append(consume(jc, ps))
                return outs

            def copy_out(tagp):
                def f(jc, ps):
                    t = hp.tile([128, BPC], f32, tag=f"{tagp}{jc}")
                    nc.vector.tensor_copy(t[:], ps[:])
                    return t
                return f

            def relu_out(tagp, bk):
                def f(jc, ps):
                    t = hp.tile([128, BPC], f32, tag=f"{tagp}{jc}")
                    nc.scalar.activation(t[:], ps[:], AF.Relu, bias=bcol(bk, jc))
                    return t
                return f

            # ---- fused_mental; tom_hp pre-sigmoid ----
            fm_sb = proj512(wtf, [t[:] for t in fused_sb], copy_out("fm"))
            s3_ps = ptiny.tile([1, 24], f32, tag="s3")
            for c in range(4):
                nc.tensor.matmul(s3_ps[:, 0:8], vcol(0, c), fm_sb[c][:],
                                 start=(c == 0), stop=(c == 3))

            # ---- GCACU ----
            h1_sb = proj512(wg1, [cview(c, 8) for c in range(4)], relu_out("h1", 0))
            ctx_sb = proj512(wg2, [t[:] for t in h1_sb], copy_out("ctxr"))
            ctxb_sb = []
            for jc in range(4):
                t = hp.tile([128, BPC], f32, tag=f"ctx{jc}")
                nc.vector.tensor_scalar_add(t[:], ctx_sb[jc][:], bcol(1, jc))
                ctxb_sb.append(t)
            for c in range(4):
                nc.tensor.matmul(s3_ps[:, 8:16], vcol(1, c), ctxb_sb[c][:],
                                 start=(c == 0), stop=(c == 3))

            # ---- CLoST ----
            c1_sb = []
            for jc in range(4):
                ps = pj.tile([128, BPC], f32, tag="proj")
                for cc in range(8):
                    rhs = cview(cc, 9) if cc < 4 else cview(cc - 4, 10)
                    nc.tensor.matmul(
                        ps[:], wc1[:, cc * 512 + jc * 128: cc * 512 + jc * 128 + 128],
                        rhs, start=(cc == 0), stop=(cc == 7))
                t = hp.tile([128, BPC], f32, tag=f"hc{jc}")
                nc.scalar.activation(t[:], ps[:], AF.Relu, bias=bcol(2, jc))
                c1_sb.append(t)
            for c in range(4):
                nc.tensor.matmul(s3_ps[:, 16:24], vcol(2, c), c1_sb[c][:],
                                 start=(c == 0), stop=(c == 3))
            clost_sb = []
            for c in range(4):
                t = htmp.tile([128, BPC], f32, tag="cladd")
                nc.vector.tensor_add(t[:], cview(c, 9), cview(c, 10))
                t2 = hp.tile([128, BPC], f32, tag=f"cl{c}")
                nc.vector.tensor_scalar_mul(t2[:], t[:], 0.5)
                clost_sb.append(t2)

            # ---- scores3: add scalar biases, sigmoid ----
            s3b_sb = hp.tile([1, 24], f32, tag="s3b")
            nc.vector.tensor_scalar_add(s3b_sb[:, 0:8], s3_ps[:, 0:8], b5[:, 0:1])
            nc.vector.tensor_scalar_add(s3b_sb[:, 8:16], s3_ps[:, 8:16], b5[:, 1:2])
            nc.vector.tensor_scalar_add(s3b_sb[:, 16:24], s3_ps[:, 16:24], b5[:, 2:3])
            s3_sb = hp.tile([1, 24], f32, tag="s3s")
            nc.scalar.activation(s3_sb[:], s3b_sb[:], AF.Sigmoid)

            # scores3T [3, 8] via double transpose
            sbt_ps = pj.tile([128, BPC], f32, tag="proj")
            for t in range(3):
                nc.tensor.transpose(sbt_ps[0:8, t:t + 1],
                                    s3_sb[:, t * 8:(t + 1) * 8], id32_sb[:1, :1])
            sbt_sb = hp.tile([8, 3], f32, tag="sbt")
            nc.vector.tensor_copy(sbt_sb[:], sbt_ps[0:8, 0:3])
            s3t_ps = pj.tile([128, BPC], f32, tag="proj")
            nc.tensor.transpose(s3t_ps[0:3, 0:8], sbt_sb[:], id32_sb[:8, :8])
            s3t_sb = hp.tile([3, 8], f32, tag="s3t")
            nc.vector.tensor_copy(s3t_sb[:], s3t_ps[0:3, 0:8])

            # ---- mHC mix + unit-norm + mean over streams ----
            m3bc = hp.tile([128, 9], f32, tag="m3bc")
            nc.gpsimd.partition_broadcast(m3bc[:], m3[:])
            streams = [fm_sb, ctxb_sb, clost_sb]
            ss_ps = ptiny.tile([1, 24], f32, tag="ss")
            mx = [[None] * 4 for _ in range(3)]
            for i in range(3):
                for c in range(4):
                    a = htmp.tile([128, BPC], f32, tag="mxa")
                    nc.vector.tensor_scalar_mul(a[:], streams[0][c][:],
                                                m3bc[:, i * 3:i * 3 + 1])
                    bb = htmp.tile([128, BPC], f32, tag="mxb")
                    nc.vector.scalar_tensor_tensor(
                        bb[:], streams[1][c][:], m3bc[:, i * 3 + 1:i * 3 + 2],
                        a[:], ALU.mult, ALU.add)
                    m_t = hp.tile([128, BPC], f32, tag=f"mx{i}{c}")
                    nc.vector.scalar_tensor_tensor(
                        m_t[:], streams[2][c][:], m3bc[:, i * 3 + 2:i * 3 + 3],
                        bb[:], ALU.mult, ALU.add)
                    mx[i][c] = m_t
                    sq = htmp.tile([128, BPC], f32, tag="sq")
                    nc.vector.tensor_mul(sq[:], m_t[:], m_t[:])
                    nc.tensor.matmul(ss_ps[:, i * 8:(i + 1) * 8], ones_sb[:],
                                     sq[:], start=(c == 0), stop=(c == 3))
            nrm_sb = hp.tile([1, 24], f32, tag="nrm")
            nc.scalar.activation(nrm_sb[:], ss_ps[:], AF.Sqrt)
            nrm2_sb = hp.tile([1, 24], f32, tag="nrm2")
            nc.vector.tensor_scalar_add(nrm2_sb[:], nrm_sb[:], 1e-6)
            inv_sb = hp.tile([1, 24], f32, tag="inv")
            nc.vector.reciprocal(inv_sb[:], nrm2_sb[:])
            inv3_sb = hp.tile([1, 24], f32, tag="inv3")
            nc.vector.tensor_scalar_mul(inv3_sb[:], inv_sb[:], 1.0 / 3.0)
            invbc = hp.tile([128, 24], f32, tag="invbc")
            nc.gpsimd.partition_broadcast(invbc[:], inv3_sb[:])
            pmix_sb = []
            for c in range(4):
                p0 = htmp.tile([128, BPC], f32, tag="pm0")
                nc.vector.tensor_mul(p0[:], mx[0][c][:], invbc[:, 0:8])
                p1 = htmp.tile([128, BPC], f32, tag="pm1")
                nc.vector.tensor_mul(p1[:], mx[1][c][:], invbc[:, 8:16])
                p01 = htmp.tile([128, BPC], f32, tag="pm01")
                nc.vector.tensor_add(p01[:], p0[:], p1[:])
                p2 = htmp.tile([128, BPC], f32, tag="pm2")
                nc.vector.tensor_mul(p2[:], mx[2][c][:], invbc[:, 16:24])
                pm = hp.tile([128, BPC], f32, tag=f"pmix{c}")
                nc.vector.tensor_add(pm[:], p01[:], p2[:])
                pmix_sb.append(pm)

            # ---- SEVADE + final head ----
            fin_ps = ptiny.tile([1, 16], f32, tag="fin")
            for (w_main, w_tail, vk, bk, col) in (
                    (ws1, ws1t, 3, 3, 0), (wf1, wf1t, 4, 4, 8)):
                for jc in range(4):
                    ps = pj.tile([128, BPC], f32, tag="proj")
                    for c in range(4):
                        nc.tensor.matmul(
                            ps[:],
                            w_main[:, c * D + jc * 128: c * D + jc * 128 + 128],
                            pmix_sb[c][:], start=(c == 0), stop=False)
                    nc.tensor.matmul(ps[:], w_tail[:, jc * 128: jc * 128 + 128],
                                     s3t_sb[:], start=False, stop=True)
                    hs = htmp.tile([128, BPC], f32, tag="hs")
                    nc.scalar.activation(hs[:], ps[:], AF.Relu, bias=bcol(bk, jc))
                    nc.tensor.matmul(fin_ps[:, col:col + 8], vcol(vk, jc), hs[:],
                                     start=(jc == 0), stop=(jc == 3))

            # ---- combine: fin + 0.5*sev + 0.1*safe_logit(mean(s3)) ----
            sev_l = hp.tile([1, 8], f32, tag="sevl")
            nc.vector.tensor_scalar_add(sev_l[:], fin_ps[:, 0:8], b5[:, 3:4])
            fin_l = hp.tile([1, 8], f32, tag="finl")
            nc.vector.tensor_scalar_add(fin_l[:], fin_ps[:, 8:16], b5[:, 4:5])
            t1 = hp.tile([1, 8], f32, tag="t1")
            nc.vector.tensor_add(t1[:], s3_sb[:, 0:8], s3_sb[:, 8:16])
            t2 = hp.tile([1, 8], f32, tag="t2")
            nc.vector.tensor_add(t2[:], t1[:], s3_sb[:, 16:24])
            pm3 = hp.tile([1, 8], f32, tag="pm3")
            nc.vector.tensor_scalar_mul(pm3[:], t2[:], 1.0 / 3.0)
            pcl = hp.tile([1, 8], f32, tag="pcl")
            nc.vector.tensor_scalar(pcl[:], pm3[:], EPS, 1.0 - EPS,
                                    ALU.max, ALU.min)
            lp = hp.tile([1, 8], f32, tag="lp")
            nc.scalar.activation(lp[:], pcl[:], AF.Ln)
            omp = hp.tile([1, 8], f32, tag="omp")
            nc.vector.tensor_scalar(omp[:], pcl[:], -1.0, 1.0, ALU.mult, ALU.add)
            l1p = hp.tile([1, 8], f32, tag="l1p")
            nc.scalar.activation(l1p[:], omp[:], AF.Ln)
            lg = hp.tile([1, 8], f32, tag="lg")
            nc.vector.tensor_sub(lg[:], lp[:], l1p[:])
            o1 = hp.tile([1, 8], f32, tag="o1")
            nc.vector.scalar_tensor_tensor(o1[:], sev_l[:], 0.5, fin_l[:],
                                           ALU.mult, ALU.add)
            o2 = hp.tile([1, 8], f32, tag="o2")
            nc.vector.scalar_tensor_tensor(o2[:], lg[:], 0.1, o1[:],
                                           ALU.mult, ALU.add)
            nc.sync.dma_start(out=out_d[:], in_=o2[:])

    nc.compile()
    return nc


def _pack_w(w, ncol=512):
    w = np.asarray(w, np.float32)
    nchunk = w.shape[0] // 128
    return np.ascontiguousarray(
        w.reshape(nchunk, 128, ncol).transpose(1, 0, 2).reshape(128, nchunk * ncol))


def _pack_v(v):
    v = np.asarray(v, np.float32).reshape(-1)
    return np.ascontiguousarray(v.reshape(4, 128).T)


def _prep_host(inputs):
    f8 = np.float64
    Wk = np.asarray(inputs["Wk"], f8)
    q_tom = np.asarray(inputs["q_tom"], f8)
    qk = np.einsum("dhk,hk->dh", Wk.reshape(D, NH, DH), q_tom) / np.sqrt(
        np.float64(DH))
    import ml_dtypes
    bf = ml_dtypes.bfloat16
    qk_full = np.zeros((D, 11), np.float32)
    qk_full[:, :NH] = qk.astype(np.float32)
    qk_pk = np.ascontiguousarray(
        qk_full.reshape(4, 128, 11).transpose(1, 0, 2).reshape(128, 44)).astype(bf)

    m = np.asarray(inputs["attention_mask"], f8)  # [B, S]
    cum = np.cumsum(m, axis=1)
    valid = cum[:, -1:]
    split = np.maximum(1.0, np.floor(valid * 0.6))
    setup = m * (cum <= split)
    punch = m * (cum > split)
    pc = punch.sum(1, keepdims=True)
    last = m * (cum == valid)
    punch = np.where(pc > 0, punch, last)

    def logpre(msk):
        s = msk.sum(1, keepdims=True)
        pre = msk / s
        out = np.full(pre.shape, LOG_FLOOR, f8)
        np.log(pre, out=out, where=pre > 0)
        return out

    row0 = MASK_NEG * (1.0 - m)
    rows = np.stack([row0, logpre(m), logpre(setup), logpre(punch)], 0)  # [4,B,S]
    rhs4 = rows.astype(np.float32).astype(bf)

    sel = np.zeros((4, 11), bf)
    sel[0, :8] = 1.0
    sel[1, 8] = 1.0
    sel[2, 9] = 1.0
    sel[3, 10] = 1.0

    M3 = (np.eye(3, dtype=f8)
          + np.asarray(inputs["U_mhc"], f8) @ np.asarray(inputs["V_mhc"], f8))
    m3 = np.ascontiguousarray(M3.astype(np.float32).reshape(1, 9))

    Ws1 = np.asarray(inputs["Ws1"], np.float32)
    Wf1 = np.asarray(inputs["Wf1"], np.float32)
    vecs = np.concatenate([
        _pack_v(inputs["w_hp"]), _pack_v(inputs["w_inc"]), _pack_v(inputs["wc2"]),
        _pack_v(inputs["ws2"]), _pack_v(inputs["wf2"])], axis=1)
    bvecs = np.concatenate([
        _pack_v(inputs["bg1"]), _pack_v(inputs["bg2"]), _pack_v(inputs["bc1"]),
        _pack_v(inputs["bs1"]), _pack_v(inputs["bf1"])], axis=1)
    b5 = np.array([[np.float32(np.asarray(inputs[k]).reshape(-1)[0])
                    for k in ("b_hp", "b_inc", "bc2", "bs2", "bf2")]], np.float32)

    shared = {
        "qk": qk_pk, "sel": sel, "ident": np.eye(128, dtype=np.float32).astype(bf),
        "ident32": np.eye(16, dtype=np.float32),
        "wv": _pack_w(inputs["Wv"]), "wtf": _pack_w(inputs["W_tom_fuse"]),
        "wg1": _pack_w(inputs["Wg1"]), "wg2": _pack_w(inputs["Wg2"]),
        "wc1": _pack_w(inputs["Wc1"]),
        "ws1": _pack_w(Ws1[:512]), "ws1t": np.ascontiguousarray(Ws1[512:515]),
        "wf1": _pack_w(Wf1[:512]), "wf1t": np.ascontiguousarray(Wf1[512:515]),
        "vecs": np.ascontiguousarray(vecs), "bvecs": np.ascontiguousarray(bvecs),
        "b5": b5, "m3": m3,
    }
    x = np.asarray(inputs["embeddings"], np.float32).astype(bf)
    in_maps = []
    for k in range(NCORES):
        d = dict(shared)
        d["x"] = np.ascontiguousarray(x[k * BPC:(k + 1) * BPC])
        d["rhs4"] = np.ascontiguousarray(
            rhs4[:, k * BPC:(k + 1) * BPC].reshape(4, BPC * S))
        in_maps.append(d)
    return in_maps


def _install_ntff_shim():
    """antenv.axon_hooks is absent in this image; recreate it so
    run_bass_kernel_spmd(trace=True) can capture NTFF profiles."""
    import sys
    import types
    if "antenv.axon_hooks" in sys.modules:
        return
    mod = types.ModuleType("antenv.axon_hooks")
    mod._hook = None
    mod.set_axon_ntff_profile_hook = lambda h: setattr(mod, "_hook", h)
    mod.get_axon_ntff_profile_hook = lambda: mod._hook
    sys.modules["antenv.axon_hooks"] = mod
    try:
        import antenv
        antenv.axon_hooks = mod
        from trn_agent_boot.trn_boot import _ntff_profile_via_ctypes
        mod._hook = _ntff_profile_via_ctypes("/opt/axon/libaxon_pjrt.so")
    except Exception as e:
        print(f"ntff shim setup failed ({e}); tracing disabled")


def kernel(**inputs):
    global LAST_RESULT
    _install_ntff_shim()
    from concourse.bass_utils import run_bass_kernel_spmd

    if "nc" not in _CACHE:
        _CACHE["nc"] = _build_program()
    nc = _CACHE["nc"]

    in_maps = _prep_host(inputs)
    trace = os.environ.get("BASS_TRACE", "0") == "1"
    res = run_bass_kernel_spmd(nc, in_maps, list(range(NCORES)), trace=trace)
    LAST_RESULT = res
    out = np.empty((B, 1), np.float32)
    for k in range(NCORES):
        out[k * BPC:(k + 1) * BPC, 0] = np.asarray(res.results[k]["out"]).reshape(-1)
    return out


# revision 8
# speedup vs baseline: 1.0336x; 1.0336x over previous
"""Trainium2 Bass kernel for nn_IntegratedLaughterModel.

Strategy (pure data parallel, 8 samples/core):
  - Algebraic reduction: scores[b,h,s] = x[b,s,:] @ qk[:,h] where
    qk = (Wk reshaped) @ q_tom / sqrt(DH)  (host-precomputed [D, NH]).
    This removes the two [S,D]x[D,D] matmuls entirely.
  - Single pass over x per core: per 512-token group,
      PE-transpose x chunks -> scores matmul (fp32r) -> mask via rank-4
      log-mask matmul -> ACT exp (with accum Z) -> PE-transpose weights ->
      pooling matmul accumulating [11, D] per sample.
    rows 0..7  = unnormalized attn-weighted sums per head,
    rows 8..10 = masked means (mean/setup/punch; masks pre-scaled on host,
                 folded in as exp(log(mask)) rows so one pooling matmul).
  - Small per-core head (feature-major, [128d, 8b] tiles) computes the
    ToM/GCACU/CLoST/mHC/SEVADE/final MLPs on-device.
"""

import os
import numpy as np

B, S, D, HID, NH = 64, 2048, 512, 512, 8
DH = D // NH
NCORES = 8
BPC = B // NCORES   # samples per core
NG = 4              # 512-token groups per sample
GT = 512            # tokens per group
NT = 4              # 128-token subtiles per group
NCD = 4             # d-chunks of 128
EPS = 1e-4
MASK_NEG = -30.0    # additive penalty for masked tokens (exp(-30) ~ 1e-13)
LOG_FLOOR = -80.0

F32R = os.environ.get("KERNEL_F32R", "1") == "1"

_CACHE = {}
LAST_RESULT = None


def _build_program():
    import concourse.bacc as bacc
    import concourse.tile as tile
    from concourse import mybir
    from contextlib import ExitStack

    f32 = mybir.dt.float32
    bf16 = mybir.dt.bfloat16
    AF = mybir.ActivationFunctionType
    ALU = mybir.AluOpType

    nc = bacc.Bacc("TRN2", target_bir_lowering=False, debug=False,
                   enable_asserts=False)

    # ---- DRAM I/O ----
    x_d = nc.dram_tensor("x", [BPC, S, D], bf16, kind="ExternalInput").ap()
    rhs4_d = nc.dram_tensor("rhs4", [4, BPC * NG * GT], bf16, kind="ExternalInput").ap()
    qk_d = nc.dram_tensor("qk", [128, NCD * 11], bf16, kind="ExternalInput").ap()
    sel_d = nc.dram_tensor("sel", [4, 11], bf16, kind="ExternalInput").ap()
    id_d = nc.dram_tensor("ident", [128, 128], bf16, kind="ExternalInput").ap()
    id32_d = nc.dram_tensor("ident32", [16, 16], f32, kind="ExternalInput").ap()
    wv_d = nc.dram_tensor("wv", [128, 2048], f32, kind="ExternalInput").ap()
    wtf_d = nc.dram_tensor("wtf", [128, 2048], f32, kind="ExternalInput").ap()
    wg1_d = nc.dram_tensor("wg1", [128, 2048], f32, kind="ExternalInput").ap()
    wg2_d = nc.dram_tensor("wg2", [128, 2048], f32, kind="ExternalInput").ap()
    wc1_d = nc.dram_tensor("wc1", [128, 4096], f32, kind="ExternalInput").ap()
    ws1_d = nc.dram_tensor("ws1", [128, 2048], f32, kind="ExternalInput").ap()
    ws1t_d = nc.dram_tensor("ws1t", [3, 512], f32, kind="ExternalInput").ap()
    wf1_d = nc.dram_tensor("wf1", [128, 2048], f32, kind="ExternalInput").ap()
    wf1t_d = nc.dram_tensor("wf1t", [3, 512], f32, kind="ExternalInput").ap()
    vecs_d = nc.dram_tensor("vecs", [128, 20], f32, kind="ExternalInput").ap()
    bvecs_d = nc.dram_tensor("bvecs", [128, 20], f32, kind="ExternalInput").ap()
    b5_d = nc.dram_tensor("b5", [1, 5], f32, kind="ExternalInput").ap()
    m3_d = nc.dram_tensor("m3", [1, 9], f32, kind="ExternalInput").ap()
    out_d = nc.dram_tensor("out", [1, BPC], f32, kind="ExternalOutput").ap()
    diag_d = nc.dram_tensor("diag", [BPC * 11, D], f32, kind="ExternalOutput").ap()

    with tile.TileContext(nc) as tc, ExitStack() as ctx:
        cst = ctx.enter_context(tc.tile_pool(name="cst", bufs=1))

        def static(name, shape, src, dt=f32):
            t = cst.tile(shape, dt, tag=name, name=name)
            nc.sync.dma_start(out=t[:], in_=src)
            return t

        qk_sb = static("qk", [128, NCD * 11], qk_d, bf16)
        sel_sb = static("sel", [4, 11], sel_d, bf16)
        id_sb = static("ident", [128, 128], id_d, bf16)
        id32_sb = static("ident32", [16, 16], id32_d, f32)
        rhs4_sb = static("rhs4", [4, BPC * NG * GT], rhs4_d, bf16)
        ones_sb = cst.tile([128, 1], f32, tag="ones")
        nc.vector.memset(ones_sb[:], 1.0)
        ones16_sb = cst.tile([128, 1], bf16, tag="ones16")
        nc.vector.memset(ones16_sb[:], 1.0)

        # pooledT chunks: [128 d, BPC*11] feature-major pooled quantities
        pT = [cst.tile([128, BPC * 11], f32, tag=f"pT{c}", name=f"pT{c}")
              for c in range(NCD)]

        H = {}

        def load_head_weights():
            H["wv"] = static("wv", [128, 2048], wv_d)
            H["wtf"] = static("wtf", [128, 2048], wtf_d)
            H["wg1"] = static("wg1", [128, 2048], wg1_d)
            H["wg2"] = static("wg2", [128, 2048], wg2_d)
            H["wc1"] = static("wc1", [128, 4096], wc1_d)
            H["ws1"] = static("ws1", [128, 2048], ws1_d)
            H["ws1t"] = static("ws1t", [3, 512], ws1t_d)
            H["wf1"] = static("wf1", [128, 2048], wf1_d)
            H["wf1t"] = static("wf1t", [3, 512], wf1t_d)
            H["vecs"] = static("vecs", [128, 20], vecs_d)
            H["bvecs"] = static("bvecs", [128, 20], bvecs_d)
            H["b5"] = static("b5", [1, 5], b5_d)
            H["m3"] = static("m3", [1, 9], m3_d)

        # ================= main streaming pass =================
        with ExitStack() as pctx:
            xg_p = pctx.enter_context(tc.tile_pool(name="xg", bufs=6))
            xt_ps_p = pctx.enter_context(tc.tile_pool(name="xtps", bufs=2, space="PSUM"))
            xt_sb_p = pctx.enter_context(tc.tile_pool(name="xtsb", bufs=5))
            sc_ps_p = pctx.enter_context(tc.tile_pool(name="scps", bufs=2, space="PSUM"))
            w_sb_p = pctx.enter_context(tc.tile_pool(name="wsb", bufs=3))
            wt_ps_p = pctx.enter_context(tc.tile_pool(name="wtps", bufs=1, space="PSUM"))
            wt_sb_p = pctx.enter_context(tc.tile_pool(name="wtsb", bufs=3))
            pool_ps_p = pctx.enter_context(tc.tile_pool(name="poolps", bufs=2, space="PSUM"))
            z_ps_p = pctx.enter_context(tc.tile_pool(name="zps", bufs=1, space="PSUM"))
            small_p = pctx.enter_context(tc.tile_pool(name="small", bufs=3))

            for b in range(BPC):
                pool_ps = pool_ps_p.tile([11, D], f32, tag="pool")
                z_ps = z_ps_p.tile([11, 1], f32, tag="z")
                for g in range(NG):
                    xg = xg_p.tile([128, NT * D], bf16, tag="xg")
                    src = x_d[b, g * GT:(g + 1) * GT, :].rearrange(
                        "(t p) d -> p t d", p=128)
                    nc.sync.dma_start(
                        out=xg[:].rearrange("p (t d) -> p t d", d=D), in_=src)

                    sc_ps = sc_ps_p.tile([11, GT], f32, tag="sc")
                    # transpose x chunks and accumulate scores over d-chunks
                    for c in range(NCD):
                        xt_ps = xt_ps_p.tile([128, GT], bf16, tag="xt")
                        for t in range(NT):
                            nc.tensor.transpose(
                                xt_ps[:, t * 128:(t + 1) * 128],
                                xg[:, t * D + c * 128: t * D + (c + 1) * 128],
                                id_sb[:])
                        xt_sb = xt_sb_p.tile([128, GT], bf16, tag="xt")
                        nc.vector.tensor_copy(xt_sb[:], xt_ps[:])
                        nc.tensor.matmul(
                            sc_ps[:], qk_sb[:, c * 11:(c + 1) * 11],
                            xt_sb[:], start=(c == 0), stop=False)
                    # rank-4 mask/log-mask rows
                    col0 = (b * NG + g) * GT
                    nc.tensor.matmul(
                        sc_ps[:], sel_sb[:],
                        rhs4_sb[:, col0:col0 + GT], start=False, stop=True)
                    # exp
                    w_sb = w_sb_p.tile([11, GT], f32, tag="w")
                    nc.scalar.activation(w_sb[:], sc_ps[:], AF.Exp)
                    # transpose w -> [128, 11] per subtile (cast to bf16); pool
                    wt_ps = wt_ps_p.tile([128, NT * 11], f32, tag="wt")
                    for t in range(NT):
                        nc.tensor.transpose(
                            wt_ps[:, t * 11:(t + 1) * 11],
                            w_sb[:, t * 128:(t + 1) * 128],
                            id32_sb[:11, :11])
                    wt_sb = wt_sb_p.tile([128, NT * 11], bf16, tag="wt")
                    nc.vector.tensor_copy(wt_sb[:], wt_ps[:])
                    for t in range(NT):
                        nc.tensor.matmul(
                            pool_ps[:], wt_sb[:, t * 11:(t + 1) * 11],
                            xg[:, t * D:(t + 1) * D],
                            start=(g == 0 and t == 0),
                            stop=(g == NG - 1 and t == NT - 1))
                        nc.tensor.matmul(
                            z_ps[:], wt_sb[:, t * 11:(t + 1) * 11],
                            ones16_sb[:],
                            start=(g == 0 and t == 0),
                            stop=(g == NG - 1 and t == NT - 1))

                # normalize by Z and extract feature-major pooledT
                zr = small_p.tile([11, 1], f32, tag="zr")
                nc.vector.reciprocal(zr[:], z_ps[:])
                pooled_sb = small_p.tile([11, D], f32, tag="pooled")
                nc.vector.tensor_scalar_mul(pooled_sb[:], pool_ps[:], zr[:])
                nc.sync.dma_start(out=diag_d[b * 11:(b + 1) * 11, :],
                                  in_=pooled_sb[:])
                for c in range(NCD):
                    pt_ps = wt_ps_p.tile([128, NT * 11], f32, tag="wt")
                    nc.tensor.transpose(pt_ps[:, 0:11],
                                        pooled_sb[:, c * 128:(c + 1) * 128],
                                        id32_sb[:11, :11])
                    nc.vector.tensor_copy(pT[c][:, b * 11:(b + 1) * 11],
                                          pt_ps[:, 0:11])

                if b == 0:
                    load_head_weights()

        # ================= head (feature-major, all 8 samples) =================
        def cview(c, r):
            """[128, 8] view of quantity r across samples in pooledT chunk c."""
            return pT[c][:].rearrange("p (b q) -> p b q", q=11)[:, :, r]

        with ExitStack() as hctx:
            pj = hctx.enter_context(tc.tile_pool(name="pj", bufs=2, space="PSUM"))
            ptiny = hctx.enter_context(tc.tile_pool(name="ptiny", bufs=1, space="PSUM"))
            hp = hctx.enter_context(tc.tile_pool(name="hp", bufs=1))
            htmp = hctx.enter_context(tc.tile_pool(name="htmp", bufs=4))

            wv = H["wv"]; wtf = H["wtf"]; wg1 = H["wg1"]; wg2 = H["wg2"]
            wc1 = H["wc1"]; ws1 = H["ws1"]; ws1t = H["ws1t"]; wf1 = H["wf1"]
            wf1t = H["wf1t"]; vecs = H["vecs"]; bvecs = H["bvecs"]
            b5 = H["b5"]; m3 = H["m3"]

            def vcol(k, c):
                return vecs[:, k * 4 + c: k * 4 + c + 1]

            def bcol(k, c):
                return bvecs[:, k * 4 + c: k * 4 + c + 1]

            # ---- fusedT = blockdiag(Wv) applied to attn-pooled heads ----
            fused_sb = []
            for i in range(4):
                ps = pj.tile([128, BPC], f32, tag="proj")
                for hh in range(2):
                    h = 2 * i + hh
                    o = ps[hh * 64:(hh + 1) * 64, :]
                    for c in range(NCD):
                        nc.tensor.matmul(
                            o, wv[:, c * D + h * DH: c * D + (h + 1) * DH],
                            cview(c, h), start=(c == 0), stop=(c == NCD - 1))
                t = hp.tile([128, BPC], f32, tag=f"fused{i}")
                nc.vector.tensor_copy(t[:], ps[:])
                fused_sb.append(t)

            def proj512(w_tile, rhs_aps, consume, nchunks=4):
                """per jc: ps[j,b] = sum_c W_chunk.T @ rhs_c; consume(jc, ps)."""
                outs = []
                for jc in range(4):
                    ps = pj.tile([128, BPC], f32, tag="proj")
                    for c in range(nchunks):
                        nc.tensor.matmul(
                            ps[:],
                            w_tile[:, c * D + jc * 128: c * D + jc * 128 + 128],
                            rhs_aps[c], start=(c == 0), stop=(c == nchunks - 1))
                    outs.append(consume(jc, ps))
                return outs

            def copy_out(tagp):
                def f(jc, ps):
                    t = hp.tile([128, BPC], f32, tag=f"{tagp}{jc}")
                    nc.vector.tensor_copy(t[:], ps[:])
                    return t
                return f

            def relu_out(tagp, bk):
                def f(jc, ps):
                    t = hp.tile([128, BPC], f32, tag=f"{tagp}{jc}")
                    nc.scalar.activation(t[:], ps[:], AF.Relu, bias=bcol(bk, jc))
                    return t
                return f

            # ---- fused_mental; tom_hp pre-sigmoid ----
            fm_sb = proj512(wtf, [t[:] for t in fused_sb], copy_out("fm"))
            s3_ps = ptiny.tile([1, 24], f32, tag="s3")
            for c in range(4):
                nc.tensor.matmul(s3_ps[:, 0:8], vcol(0, c), fm_sb[c][:],
                                 start=(c == 0), stop=(c == 3))

            # ---- GCACU ----
            h1_sb = proj512(wg1, [cview(c, 8) for c in range(4)], relu_out("h1", 0))
            ctx_sb = proj512(wg2, [t[:] for t in h1_sb], copy_out("ctxr"))
            ctxb_sb = []
            for jc in range(4):
                t = hp.tile([128, BPC], f32, tag=f"ctx{jc}")
                nc.vector.tensor_scalar_add(t[:], ctx_sb[jc][:], bcol(1, jc))
                ctxb_sb.append(t)
            for c in range(4):
                nc.tensor.matmul(s3_ps[:, 8:16], vcol(1, c), ctxb_sb[c][:],
                                 start=(c == 0), stop=(c == 3))

            # ---- CLoST ----
            c1_sb = []
            for jc in range(4):
                ps = pj.tile([128, BPC], f32, tag="proj")
                for cc in range(8):
                    rhs = cview(cc, 9) if cc < 4 else cview(cc - 4, 10)
                    nc.tensor.matmul(
                        ps[:], wc1[:, cc * 512 + jc * 128: cc * 512 + jc * 128 + 128],
                        rhs, start=(cc == 0), stop=(cc == 7))
                t = hp.tile([128, BPC], f32, tag=f"hc{jc}")
                nc.scalar.activation(t[:], ps[:], AF.Relu, bias=bcol(2, jc))
                c1_sb.append(t)
            for c in range(4):
                nc.tensor.matmul(s3_ps[:, 16:24], vcol(2, c), c1_sb[c][:],
                                 start=(c == 0), stop=(c == 3))
            clost_sb = []
            for c in range(4):
                t = htmp.tile([128, BPC], f32, tag="cladd")
                nc.vector.tensor_add(t[:], cview(c, 9), cview(c, 10))
                t2 = hp.tile([128, BPC], f32, tag=f"cl{c}")
                nc.vector.tensor_scalar_mul(t2[:], t[:], 0.5)
                clost_sb.append(t2)

            # ---- scores3: add scalar biases, sigmoid ----
            s3b_sb = hp.tile([1, 24], f32, tag="s3b")
            nc.vector.tensor_scalar_add(s3b_sb[:, 0:8], s3_ps[:, 0:8], b5[:, 0:1])
            nc.vector.tensor_scalar_add(s3b_sb[:, 8:16], s3_ps[:, 8:16], b5[:, 1:2])
            nc.vector.tensor_scalar_add(s3b_sb[:, 16:24], s3_ps[:, 16:24], b5[:, 2:3])
            s3_sb = hp.tile([1, 24], f32, tag="s3s")
            nc.scalar.activation(s3_sb[:], s3b_sb[:], AF.Sigmoid)

            # scores3T [3, 8] via double transpose
            sbt_ps = pj.tile([128, BPC], f32, tag="proj")
            for t in range(3):
                nc.tensor.transpose(sbt_ps[0:8, t:t + 1],
                                    s3_sb[:, t * 8:(t + 1) * 8], id32_sb[:1, :1])
            sbt_sb = hp.tile([8, 3], f32, tag="sbt")
            nc.vector.tensor_copy(sbt_sb[:], sbt_ps[0:8, 0:3])
            s3t_ps = pj.tile([128, BPC], f32, tag="proj")
            nc.tensor.transpose(s3t_ps[0:3, 0:8], sbt_sb[:], id32_sb[:8, :8])
            s3t_sb = hp.tile([3, 8], f32, tag="s3t")
            nc.vector.tensor_copy(s3t_sb[:], s3t_ps[0:3, 0:8])

            # ---- mHC mix + unit-norm + mean over streams ----
            m3bc = hp.tile([128, 9], f32, tag="m3bc")
            nc.gpsimd.partition_broadcast(m3bc[:], m3[:])
            streams = [fm_sb, ctxb_sb, clost_sb]
            ss_ps = ptiny.tile([1, 24], f32, tag="ss")
            mx = [[None] * 4 for _ in range(3)]
            for i in range(3):
                for c in range(4):
                    a = htmp.tile([128, BPC], f32, tag="mxa")
                    nc.vector.tensor_scalar_mul(a[:], streams[0][c][:],
                                                m3bc[:, i * 3:i * 3 + 1])
                    bb = htmp.tile([128, BPC], f32, tag="mxb")
                    nc.vector.scalar_tensor_tensor(
                        bb[:], streams[1][c][:], m3bc[:, i * 3 + 1:i * 3 + 2],
                        a[:], ALU.mult, ALU.add)
                    m_t = hp.tile([128, BPC], f32, tag=f"mx{i}{c}")
                    nc.vector.scalar_tensor_tensor(
                        m_t[:], streams[2][c][:], m3bc[:, i * 3 + 2:i * 3 + 3],
                        bb[:], ALU.mult, ALU.add)
                    mx[i][c] = m_t
                    sq = htmp.tile([128, BPC], f32, tag="sq")
                    nc.vector.tensor_mul(sq[:], m_t[:], m_t[:])
                    nc.tensor.matmul(ss_ps[:, i * 8:(i + 1) * 8], ones_sb[:],
                                     sq[:], start=(c == 0), stop=(c == 3))
            nrm_sb = hp.tile([1, 24], f32, tag="nrm")
            nc.scalar.activation(nrm_sb[:], ss_ps[:], AF.Sqrt)
            nrm2_sb = hp.tile([1, 24], f32, tag="nrm2")
            nc.vector.tensor_scalar_add(nrm2_sb[:], nrm_sb[:], 1e-6)
            inv_sb = hp.tile([1, 24], f32, tag="inv")
            nc.vector.reciprocal(inv_sb[:], nrm2_sb[:])
            inv3_sb = hp.tile([1, 24], f32, tag="inv3")
            nc.vector.tensor_scalar_mul(inv3_sb[:], inv_sb[:], 1.0 / 3.0)
            invbc = hp.tile([128, 24], f32, tag="invbc")
            nc.gpsimd.partition_broadcast(invbc[:], inv3_sb[:])
            pmix_sb = []
            for c in range(4):
                p0 = htmp.tile([128, BPC], f32, tag="pm0")
                nc.vector.tensor_mul(p0[:], mx[0][c][:], invbc[:, 0:8])
                p1 = htmp.tile([128, BPC], f32, tag="pm1")
                nc.vector.tensor_mul(p1[:], mx[1][c][:], invbc[:, 8:16])
                p01 = htmp.tile([128, BPC], f32, tag="pm01")
                nc.vector.tensor_add(p01[:], p0[:], p1[:])
                p2 = htmp.tile([128, BPC], f32, tag="pm2")
                nc.vector.tensor_mul(p2[:], mx[2][c][:], invbc[:, 16:24])
                pm = hp.tile([128, BPC], f32, tag=f"pmix{c}")
                nc.vector.tensor_add(pm[:], p01[:], p2[:])
                pmix_sb.append(pm)

            # ---- SEVADE + final head ----
            fin_ps = ptiny.tile([1, 16], f32, tag="fin")
            for (w_main, w_tail, vk, bk, col) in (
                    (ws1, ws1t, 3, 3, 0), (wf1, wf1t, 4, 4, 8)):
                for jc in range(4):
                    ps = pj.tile([128, BPC], f32, tag="proj")
                    for c in range(4):
                        nc.tensor.matmul(
                            ps[:],
                            w_main[:, c * D + jc * 128: c * D + jc * 128 + 128],
                            pmix_sb[c][:], start=(c == 0), stop=False)
                    nc.tensor.matmul(ps[:], w_tail[:, jc * 128: jc * 128 + 128],
                                     s3t_sb[:], start=False, stop=True)
                    hs = htmp.tile([128, BPC], f32, tag="hs")
                    nc.scalar.activation(hs[:], ps[:], AF.Relu, bias=bcol(bk, jc))
                    nc.tensor.matmul(fin_ps[:, col:col + 8], vcol(vk, jc), hs[:],
                                     start=(jc == 0), stop=(jc == 3))

            # ---- combine: fin + 0.5*sev + 0.1*safe_logit(mean(s3)) ----
            sev_l = hp.tile([1, 8], f32, tag="sevl")
            nc.vector.tensor_scalar_add(sev_l[:], fin_ps[:, 0:8], b5[:, 3:4])
            fin_l = hp.tile([1, 8], f32, tag="finl")
            nc.vector.tensor_scalar_add(fin_l[:], fin_ps[:, 8:16], b5[:, 4:5])
            t1 = hp.tile([1, 8], f32, tag="t1")
            nc.vector.tensor_add(t1[:], s3_sb[:, 0:8], s3_sb[:, 8:16])
            t2 = hp.tile([1, 8], f32, tag="t2")
            nc.vector.tensor_add(t2[:], t1[:], s3_sb[:, 16:24])
            pm3 = hp.tile([1, 8], f32, tag="pm3")
            nc.vector.tensor_scalar_mul(pm3[:], t2[:], 1.0 / 3.0)
            pcl = hp.tile([1, 8], f32, tag="pcl")
            nc.vector.tensor_scalar(pcl[:], pm3[:], EPS, 1.0 - EPS,
                                    ALU.max, ALU.min)
            lp = hp.tile([1, 8], f32, tag="lp")
            nc.scalar.activation(lp[:], pcl[:], AF.Ln)
            omp = hp.tile([1, 8], f32, tag="omp")
            nc.vector.tensor_scalar(omp[:], pcl[:], -1.0, 1.0, ALU.mult, ALU.add)
            l1p = hp.tile([1, 8], f32, tag="l1p")
            nc.scalar.activation(l1p[:], omp[:], AF.Ln)
            lg = hp.tile([1, 8], f32, tag="lg")
            nc.vector.tensor_sub(lg[:], lp[:], l1p[:])
            o1 = hp.tile([1, 8], f32, tag="o1")
            nc.vector.scalar_tensor_tensor(o1[:], sev_l[:], 0.5, fin_l[:],
                                           ALU.mult, ALU.add)
            o2 = hp.tile([1, 8], f32, tag="o2")
            nc.vector.scalar_tensor_tensor(o2[:], lg[:], 0.1, o1[:],
                                           ALU.mult, ALU.add)
            nc.sync.dma_start(out=out_d[:], in_=o2[:])

    nc.compile()
    return nc


def _pack_w(w, ncol=512):
    w = np.asarray(w, np.float32)
    nchunk = w.shape[0] // 128
    return np.ascontiguousarray(
        w.reshape(nchunk, 128, ncol).transpose(1, 0, 2).reshape(128, nchunk * ncol))


def _pack_v(v):
    v = np.asarray(v, np.float32).reshape(-1)
    return np.ascontiguousarray(v.reshape(4, 128).T)


def _prep_host(inputs):
    f8 = np.float64
    Wk = np.asarray(inputs["Wk"], f8)
    q_tom = np.asarray(inputs["q_tom"], f8)
    qk = np.einsum("dhk,hk->dh", Wk.reshape(D, NH, DH), q_tom) / np.sqrt(
        np.float64(DH))
    import ml_dtypes
    bf = ml_dtypes.bfloat16
    qk_full = np.zeros((D, 11), np.float32)
    qk_full[:, :NH] = qk.astype(np.float32)
    qk_pk = np.ascontiguousarray(
        qk_full.reshape(4, 128, 11).transpose(1, 0, 2).reshape(128, 44)).astype(bf)

    m = np.asarray(inputs["attention_mask"], f8)  # [B, S]
    cum = np.cumsum(m, axis=1)
    valid = cum[:, -1:]
    split = np.maximum(1.0, np.floor(valid * 0.6))
    setup = m * (cum <= split)
    punch = m * (cum > split)
    pc = punch.sum(1, keepdims=True)
    last = m * (cum == valid)
    punch = np.where(pc > 0, punch, last)

    def logpre(msk):
        s = msk.sum(1, keepdims=True)
        pre = msk / s
        out = np.full(pre.shape, LOG_FLOOR, f8)
        np.log(pre, out=out, where=pre > 0)
        return out

    row0 = MASK_NEG * (1.0 - m)
    rows = np.stack([row0, logpre(m), logpre(setup), logpre(punch)], 0)  # [4,B,S]
    rhs4 = rows.astype(np.float32).astype(bf)

    sel = np.zeros((4, 11), bf)
    sel[0, :8] = 1.0
    sel[1, 8] = 1.0
    sel[2, 9] = 1.0
    sel[3, 10] = 1.0

    M3 = (np.eye(3, dtype=f8)
          + np.asarray(inputs["U_mhc"], f8) @ np.asarray(inputs["V_mhc"], f8))
    m3 = np.ascontiguousarray(M3.astype(np.float32).reshape(1, 9))

    Ws1 = np.asarray(inputs["Ws1"], np.float32)
    Wf1 = np.asarray(inputs["Wf1"], np.float32)
    vecs = np.concatenate([
        _pack_v(inputs["w_hp"]), _pack_v(inputs["w_inc"]), _pack_v(inputs["wc2"]),
        _pack_v(inputs["ws2"]), _pack_v(inputs["wf2"])], axis=1)
    bvecs = np.concatenate([
        _pack_v(inputs["bg1"]), _pack_v(inputs["bg2"]), _pack_v(inputs["bc1"]),
        _pack_v(inputs["bs1"]), _pack_v(inputs["bf1"])], axis=1)
    b5 = np.array([[np.float32(np.asarray(inputs[k]).reshape(-1)[0])
                    for k in ("b_hp", "b_inc", "bc2", "bs2", "bf2")]], np.float32)

    shared = {
        "qk": qk_pk, "sel": sel, "ident": np.eye(128, dtype=np.float32).astype(bf),
        "ident32": np.eye(16, dtype=np.float32),
        "wv": _pack_w(inputs["Wv"]), "wtf": _pack_w(inputs["W_tom_fuse"]),
        "wg1": _pack_w(inputs["Wg1"]), "wg2": _pack_w(inputs["Wg2"]),
        "wc1": _pack_w(inputs["Wc1"]),
        "ws1": _pack_w(Ws1[:512]), "ws1t": np.ascontiguousarray(Ws1[512:515]),
        "wf1": _pack_w(Wf1[:512]), "wf1t": np.ascontiguousarray(Wf1[512:515]),
        "vecs": np.ascontiguousarray(vecs), "bvecs": np.ascontiguousarray(bvecs),
        "b5": b5, "m3": m3,
    }
    x = np.asarray(inputs["embeddings"], np.float32).astype(bf)
    in_maps = []
    for k in range(NCORES):
        d = dict(shared)
        d["x"] = np.ascontiguousarray(x[k * BPC:(k + 1) * BPC])
        d["rhs4"] = np.ascontiguousarray(
            rhs4[:, k * BPC:(k + 1) * BPC].reshape(4, BPC * S))
        in_maps.append(d)
    return in_maps


def _install_ntff_shim():
    """antenv.axon_hooks is absent in this image; recreate it so
    run_bass_kernel_spmd(trace=True) can capture NTFF profiles."""
    import sys
    import types
    if "antenv.axon_hooks" in sys.modules:
        return
    mod = types.ModuleType("antenv.axon_hooks")
    mod._hook = None
    mod.set_axon_ntff_profile_hook = lambda h: setattr(mod, "_hook", h)
    mod.get_axon_ntff_profile_hook = lambda: mod._hook
    sys.modules["antenv.axon_hooks"] = mod
    try:
        import antenv
        antenv.axon_hooks = mod
        from trn_agent_boot.trn_boot import _ntff_profile_via_ctypes
        mod._hook = _ntff_profile_via_ctypes("/opt/axon/libaxon_pjrt.so")
    except Exception as e:
        print(f"ntff shim setup failed ({e}); tracing disabled")


def kernel(**inputs):
    global LAST_RESULT
    _install_ntff_shim()
    from concourse.bass_utils import run_bass_kernel_spmd

    if "nc" not in _CACHE:
        _CACHE["nc"] = _build_program()
    nc = _CACHE["nc"]

    in_maps = _prep_host(inputs)
    trace = os.environ.get("BASS_TRACE", "0") == "1"
    res = run_bass_kernel_spmd(nc, in_maps, list(range(NCORES)), trace=trace)
    LAST_RESULT = res
    out = np.empty((B, 1), np.float32)
    for k in range(NCORES):
        out[k * BPC:(k + 1) * BPC, 0] = np.asarray(res.results[k]["out"]).reshape(-1)
    return out
